# revision 46
# baseline (speedup 1.0000x reference)
"""EquivariantEdgeConv fused Bass kernel for one TRN2 chip (8 NeuronCores).

Strategy (node-sharded scatter, edge-bucketed message passing), v4:
  - Nodes are sharded: core c owns nodes [1024c, 1024c+1024), i.e. 8
    buckets of 128 nodes each. Each core receives exactly the edges whose
    *destination* lands in its node range, grouped by 128-node bucket and
    padded per-bucket to a multiple of 128 (shared static capacity).
  - The radial MLP's sqrt + silu are replaced by an 8192-entry lookup
    table over lensq with a Mobius index warp idx = 8192*q/(q+2)
    (= 8192 - 16384/(q+2): one reciprocal + one fused scale-add on DVE).
    Each row holds h = silu(w1*len) (64 fp32) plus 1/len, so the only
    ACT functions are Copy (+ Silu/Tanh per bucket) - one table set,
    zero activation-table reloads.
  - GPSIMD runs NO tensor ops (firmware tensor ops are very slow on HW);
    it only issues the indirect gathers, and those are batched per
    BUCKET (one gather per table per bucket via [128,T] offset APs),
    amortizing the ~1us SWDGE fixed cost 9x.
  - All per-edge elementwise work is fp16 (2-byte packed, SBUF) so the
    multiplies run in the DVE's 4x TensorScalarPtr mode; the A+B path
    reduce runs as two 2x tree-folds + one short reduce. fp16 also
    roughly halves the rounding error vs bf16.
  - Per 128-edge tile: PE does the h transpose, 8 radial matmul halves
    into PSUM quarters, and 2 one-hot scatter matmuls; ACT copies the
    quarters to SBUF fp16; DVE does the TP mults/reduces and combines.
  - Per bucket, the gated output block (o3.Linear + silu / sigmoid-via-
    tanh gate) runs transposed on PE/ACT/DVE and is DMA'd out.
    Outputs concatenate across cores - no collective needed.

Self-contained: shapes/sharding hardcoded for N=8192 nodes, E=65536
edges, irreps 48x0e + 16x1o, H=64.
"""

import sys

if "/opt/trn_rl_repo" not in sys.path:
    sys.path.insert(0, "/opt/trn_rl_repo")

import numpy as np

import concourse.bacc as bacc
import concourse.bass as bass
import concourse.mybir as mybir
import concourse.tile as tile
from concourse.bass import AP
from concourse.bass_utils import run_bass_kernel_spmd

M0, M1, H = 48, 16, 64
N_NODES, N_EDGES, N_CORES = 8192, 65536, 8
NODES_PER_CORE = N_NODES // N_CORES          # 1024
BUCKETS = NODES_PER_CORE // 128              # 8 buckets of 128 nodes per core
FP = mybir.dt.float32
F16 = mybir.dt.float16
I32 = mybir.dt.int32

# path normalizations (cA..cD) and the radial-MLP 1/sqrt(H), folded into w2
CA = 1.0 / np.sqrt(M0 * 2.0)
CB = 1.0 / np.sqrt(3.0 * M1 * 2.0)
CC = 1.0 / np.sqrt(M0 * 2.0)
CD = 1.0 / np.sqrt(M1 * 2.0)
SQRT3 = float(np.sqrt(3.0))

# xtab column layout (fp16): [xs 48 | xvy 16 (device-filled) | xv 48
# m-major (m,i) | pos_hi 3 | pos_lo 3 | pad 2] = 120
XS_OFF, XVY_OFF, XV_OFF, PH_OFF, PL_OFF, XT_COLS = 0, 48, 64, 112, 115, 120

# radial lookup table: HTAB rows over q = lensq, warp w(q) = W*q/(q+K)
HTAB_N, HTAB_K = 8192, 2.0
HTAB_COLS = 72  # [h 64 | 1/len | pad]

# w2pe column layout [64, 4096] (fp16, scales folded in):
#   AB block [0, 3072): col o*64 + i, o in 48; i<48 -> A path (i), i>=48 ->
#     B path (i-48).  One DVE mult+fold+reduce covers A+B -> ms[48].
#   C block [3072, 3840): col 3072 + o*48 + i, o in 16, i in 48.
#   D block [3840, 4096): col 3840 + o*16 + i, o,i in 16.
AB_OFF, C_OFF, D_OFF = 0, 3072, 3840


def _make_w2pe(w2: np.ndarray) -> np.ndarray:
    inv_sqrt_h = 1.0 / np.sqrt(H)
    perm = np.empty(4096, np.int64)
    scale = np.empty(4096, np.float32)
    for o in range(48):
        for i in range(48):
            perm[AB_OFF + o * 64 + i] = i * 48 + o
            scale[AB_OFF + o * 64 + i] = CA * inv_sqrt_h
        for ib in range(16):
            perm[AB_OFF + o * 64 + 48 + ib] = 2304 + ib * 48 + o
            scale[AB_OFF + o * 64 + 48 + ib] = CB * SQRT3 * inv_sqrt_h
    for o in range(16):
        for i in range(48):
            perm[C_OFF + o * 48 + i] = 3072 + i * 16 + o
            scale[C_OFF + o * 48 + i] = CC * SQRT3 * inv_sqrt_h
        for i in range(16):
            perm[D_OFF + o * 16 + i] = 3840 + i * 16 + o
            scale[D_OFF + o * 16 + i] = CD * inv_sqrt_h
    return (w2[:, perm] * scale[None, :]).astype(np.float32)


def _make_htab(w1: np.ndarray) -> np.ndarray:
    """[HTAB_N, HTAB_COLS] fp32: row i covers q near q_i = K*w/(W-w) with
    w = i+0.5; cols 0:64 = silu(w1*len), col 64 = 1/len."""
    i = np.arange(HTAB_N, dtype=np.float64)
    w = i + 0.5
    q = HTAB_K * w / (HTAB_N - w)
    length = np.sqrt(q)
    z = length[:, None] * np.asarray(w1, np.float64).reshape(1, H)
    h = z / (1.0 + np.exp(-z))
    tab = np.zeros((HTAB_N, HTAB_COLS), np.float32)
    tab[:, 0:64] = h.astype(np.float32)
    tab[:, 64] = (1.0 / length).astype(np.float32)
    return tab


def _wns_block(wns: np.ndarray) -> np.ndarray:
    """[48,48] lhsT for the 1o o3.Linear on (o,m)-interleaved rows:
    lhsT[(i,m),(o,m')] = Wns[i,o] * delta(m,m') / sqrt(M1)."""
    out = np.zeros((48, 48), np.float32)
    for i in range(16):
        for m in range(3):
            for o in range(16):
                out[i * 3 + m, o * 3 + m] = wns[i, o] / np.sqrt(M1)
    return out


def _prep_edges(edge_index: np.ndarray):
    """Bucket/pad edges by destination. Returns per-core index arrays and
    the shared per-bucket tile count."""
    src, dst = edge_index[0].astype(np.int64), edge_index[1].astype(np.int64)
    gb = dst >> 7  # global bucket 0..63
    order = np.argsort(gb, kind="stable")
    src_s, dst_s, gb_s = src[order], dst[order], gb[order]
    counts = np.bincount(gb_s, minlength=64)
    cap = int(np.ceil(counts.max() / 128) * 128)
    tiles_per_bucket = cap // 128

    srcidx = np.zeros((N_CORES, BUCKETS * cap), np.int32)
    dstpos = np.zeros((N_CORES, BUCKETS * cap), np.int32)
    dstloc = np.full((N_CORES, BUCKETS * cap), 300.0, np.float32)
    starts = np.concatenate([[0], np.cumsum(counts)])
    for g in range(64):
        c, b = g >> 3, g & 7
        s, e = starts[g], starts[g + 1]
        n = e - s
        o = b * cap
        srcidx[c, o : o + n] = src_s[s:e]
        dstpos[c, o : o + n] = dst_s[s:e]
        dstloc[c, o : o + n] = (dst_s[s:e] - (g << 7)).astype(np.float32)
    # reshape to [BUCKETS*128, T]: column t = tile t's per-partition indices
    def to_cols(a):
        out = np.empty((N_CORES, BUCKETS * 128, tiles_per_bucket), a.dtype)
        for b in range(BUCKETS):
            blk = a[:, b * cap : (b + 1) * cap].reshape(N_CORES, tiles_per_bucket, 128)
            out[:, b * 128 : (b + 1) * 128, :] = blk.transpose(0, 2, 1)
        return out
    return to_cols(srcidx), to_cols(dstpos), to_cols(dstloc), tiles_per_bucket


def build_kernel(tiles_per_bucket: int, reps: int = 1) -> bass.Bass:
    nc = bacc.Bacc(None, target_bir_lowering=False, debug=False)
    d_xtab = nc.declare_dram_parameter("xtab", [N_NODES, XT_COLS], F16, isOutput=False)
    d_ptab = nc.declare_dram_parameter("ptab", [N_NODES, 4], FP, isOutput=False)
    d_htab = nc.declare_dram_parameter("htab", [HTAB_N, HTAB_COLS], FP, isOutput=False)
    T = tiles_per_bucket
    d_srcidx = nc.declare_dram_parameter("srcidx", [BUCKETS * 128, T], I32, isOutput=False)
    d_dstpos = nc.declare_dram_parameter("dstpos", [BUCKETS * 128, T], I32, isOutput=False)
    d_dstloc = nc.declare_dram_parameter("dstloc", [BUCKETS * 128, T], F16, isOutput=False)
    d_w2pe = nc.declare_dram_parameter("w2pe", [H, 4096], F16, isOutput=False)
    d_ws = nc.declare_dram_parameter("ws", [M0, M0], F16, isOutput=False)
    d_wg = nc.declare_dram_parameter("wg", [M0, M0], F16, isOutput=False)
    d_wns = nc.declare_dram_parameter("wns", [48, 48], F16, isOutput=False)
    d_identf = nc.declare_dram_parameter("identf", [128, 128], FP, isOutput=False)
    d_iota = nc.declare_dram_parameter("iota", [128, 128], F16, isOutput=False)
    d_out = nc.declare_dram_parameter("out", [NODES_PER_CORE, M0], FP, isOutput=True)

    AF = mybir.ActivationFunctionType
    ALU = mybir.AluOpType

    lp_ctx = nc.allow_low_precision(
        reason="fp16 TP reduces over <=64 terms; rel-err gate is 2e-2"
    )
    lp_ctx.__enter__()
    with tile.TileContext(nc) as tc, tc.tile_pool(name="consts", bufs=1) as cp:
        w2pe_sb = cp.tile([H, 4096], F16)
        ws_sb = cp.tile([M0, M0], F16)
        wg_sb = cp.tile([M0, M0], F16)
        wns_sb = cp.tile([48, 48], F16)
        identf_sb = cp.tile([128, 128], FP)
        iota_sb = cp.tile([128, 128], F16)
        for sb, dr in (
            (w2pe_sb, d_w2pe), (ws_sb, d_ws), (wg_sb, d_wg),
            (wns_sb, d_wns), (identf_sb, d_identf), (iota_sb, d_iota),
        ):
            nc.sync.dma_start(out=sb[:], in_=dr[:])

        with (
            tc.tile_pool(name="idx", bufs=2) as idxp,
            tc.tile_pool(name="gath", bufs=2) as gathp,
            tc.tile_pool(name="geo", bufs=2) as geop,
            tc.tile_pool(name="work", bufs=2) as workp,
            tc.tile_pool(name="msg", bufs=2) as msgp,
            tc.tile_pool(name="accs", bufs=1) as accp,
            tc.tile_pool(name="accps", bufs=1, space="PSUM") as accpp,
            tc.tile_pool(name="wps", bufs=2, space="PSUM") as wpsp,
            tc.tile_pool(name="ps_small", bufs=1, space="PSUM") as psp,
            tc.tile_pool(name="ps_node", bufs=1, space="PSUM") as pnp,
        ):
          rep_ctx = tc.For_i(0, reps, 1) if reps > 1 else None
          if rep_ctx is not None:
              rep_ctx.__enter__()
          if True:
            for b in range(BUCKETS):
                sc_s = accpp.tile([48, 128], FP, tag="acc_s")
                sc_v = accpp.tile([48, 128], FP, tag="acc_v")
                bidx_s = idxp.tile([128, T], I32, tag="idx_s")
                bidx_d = idxp.tile([128, T], I32, tag="idx_d")
                bdl = idxp.tile([128, T], F16, tag="dl")
                nc.sync.dma_start(out=bidx_s[:], in_=d_srcidx[128 * b : 128 * (b + 1), :])
                nc.sync.dma_start(out=bidx_d[:], in_=d_dstpos[128 * b : 128 * (b + 1), :])
                nc.sync.dma_start(out=bdl[:], in_=d_dstloc[128 * b : 128 * (b + 1), :])
                # ---- per-tile gathers into bucket-wide tiles (HW uses
                # only the first offset per partition, so one gather per
                # tile; compute stays bucket-batched over the slices) ----
                xgb = gathp.tile([128, XT_COLS * T], F16, tag="xgb")
                pdb = gathp.tile([128, 4 * T], FP, tag="pdb")
                for t in range(T):
                    nc.gpsimd.indirect_dma_start(
                        out=xgb[:, XT_COLS * t : XT_COLS * (t + 1)],
                        out_offset=None, in_=d_xtab[:],
                        in_offset=bass.IndirectOffsetOnAxis(
                            ap=bidx_s[:, t : t + 1], axis=0),
                    )
                    nc.gpsimd.indirect_dma_start(
                        out=pdb[:, 4 * t : 4 * (t + 1)],
                        out_offset=None, in_=d_ptab[:],
                        in_offset=bass.IndirectOffsetOnAxis(
                            ap=bidx_d[:, t : t + 1], axis=0),
                    )
                # ---- bucket-batched geometry + radial-table index ----
                vecb = geop.tile([128, 3 * T], FP, tag="vecb")
                sqb = geop.tile([128, 3 * T], FP, tag="sqb")
                qkb = geop.tile([128, T], FP, tag="qkb")
                rkb = geop.tile([128, T], FP, tag="rkb")
                hidxb = geop.tile([128, T], I32, tag="hidxb")
                y1b = geop.tile([128, 3 * T], F16, tag="y1b")
                pvyb = geop.tile([128, 48 * T], F16, tag="pvyb")
                ohb = geop.tile([128, 128 * T], F16, tag="ohb")

                def v3ap(t_ap, stride, off=0):
                    return AP(t_ap.tensor, t_ap.offset + off,
                              [t_ap.ap[0], [stride, T], [1, 3]])
                # vec = pd - (pos_hi + pos_lo) of src, in fp32
                nc.vector.tensor_tensor(
                    out=v3ap(vecb[:], 3), in0=v3ap(pdb[:], 4),
                    in1=v3ap(xgb[:], XT_COLS, PH_OFF), op=ALU.subtract,
                )
                nc.vector.tensor_tensor(
                    out=v3ap(vecb[:], 3), in0=v3ap(vecb[:], 3),
                    in1=v3ap(xgb[:], XT_COLS, PL_OFF), op=ALU.subtract,
                )
                nc.vector.tensor_tensor(
                    out=v3ap(sqb[:], 3), in0=v3ap(vecb[:], 3),
                    in1=v3ap(vecb[:], 3), op=ALU.mult,
                )
                nc.vector.reduce_sum(
                    qkb[:], sqb[:].rearrange("p (t c) -> p t c", c=3),
                    axis=mybir.AxisListType.X,
                )
                nc.vector.tensor_scalar_add(qkb[:], qkb[:], HTAB_K)
                nc.vector.reciprocal(rkb[:], qkb[:])
                hfb = geop.tile([128, T], FP, tag="hfb")
                nc.vector.tensor_scalar(
                    out=hfb[:], in0=rkb[:],
                    scalar1=-float(HTAB_N * HTAB_K), scalar2=float(HTAB_N),
                    op0=ALU.mult, op1=ALU.add,
                )
                nc.vector.tensor_scalar_max(hfb[:], hfb[:], 0.0)
                nc.vector.tensor_scalar_min(hfb[:], hfb[:], float(HTAB_N - 1))
                nc.vector.tensor_copy(hidxb[:], hfb[:])
                hgb = gathp.tile([128, HTAB_COLS * T], FP, tag="hgb")
                for t in range(T):
                    nc.gpsimd.indirect_dma_start(
                        out=hgb[:, HTAB_COLS * t : HTAB_COLS * (t + 1)],
                        out_offset=None, in_=d_htab[:],
                        in_offset=bass.IndirectOffsetOnAxis(
                            ap=hidxb[:, t : t + 1], axis=0),
                    )
                # y1 = vec / len
                invl = AP(hgb.tensor, hgb.offset + 64,
                          [hgb[:].ap[0], [HTAB_COLS, T], [0, 3]])
                nc.vector.tensor_tensor(
                    out=v3ap(y1b[:], 3), in0=v3ap(vecb[:], 3), in1=invl,
                    op=ALU.mult,
                )
                # pvy[t,(m,i)] = xv[t,(m,i)] * y1[t,m]
                xv_tmi = AP(xgb.tensor, xgb.offset + XV_OFF,
                            [xgb[:].ap[0], [XT_COLS, T], [16, 3], [1, 16]])
                y1_tmi = AP(y1b.tensor, y1b.offset,
                            [y1b[:].ap[0], [3, T], [1, 3], [0, 16]])
                pvy_tmi = AP(pvyb.tensor, pvyb.offset,
                             [pvyb[:].ap[0], [48, T], [16, 3], [1, 16]])
                nc.vector.tensor_tensor(
                    out=pvy_tmi, in0=xv_tmi, in1=y1_tmi, op=ALU.mult
                )
                # xvy[t,i] = sum_m pvy[t,(m,i)] -> xgb cols [48,64) of tile t
                pvy_tim = AP(pvyb.tensor, pvyb.offset,
                             [pvyb[:].ap[0], [48, T], [1, 16], [16, 3]])
                xvy_out = AP(xgb.tensor, xgb.offset + XVY_OFF,
                             [xgb[:].ap[0], [XT_COLS, T], [1, 16]])
                nc.vector.tensor_reduce(
                    out=xvy_out, in_=pvy_tim, axis=mybir.AxisListType.X,
                    op=ALU.add,
                )
                # one-hot for all tiles: ohb[p,(t,n)] = (iota[n] == dl[p,t])
                iota_tn = AP(iota_sb.tensor, iota_sb.offset,
                             [iota_sb[:].ap[0], [0, T], [1, 128]])
                dl_tn = AP(bdl.tensor, bdl.offset,
                           [bdl[:].ap[0], [1, T], [0, 128]])
                nc.vector.tensor_tensor(
                    out=ohb[:].rearrange("p (t n) -> p t n", n=128),
                    in0=iota_tn, in1=dl_tn, op=ALU.is_equal,
                )
                for t in range(tiles_per_bucket):
                    # ---- hT = gathered h, transposed on PE ----
                    hT_ps = psp.tile([64, 128], FP, tag="hT_ps")
                    nc.tensor.transpose(
                        hT_ps[:],
                        hgb[:, HTAB_COLS * t : HTAB_COLS * t + 64],
                        identf_sb[:],
                    )
                    hT = geop.tile([H, 128], F16, tag="hT")
                    nc.scalar.activation(hT[:], hT_ps[:], AF.Copy)
                    # ---- radial matmul in 4 quarters; PSUM -> SBUF fp16 ----
                    w_ab = workp.tile([128, 3072], F16, tag="w_ab")
                    w_cd = workp.tile([128, 1024], F16, tag="w_cd")
                    for q in range(4):
                        wq = wpsp.tile([128, 1024], FP, tag="wq")
                        for hh in range(2):
                            nc.tensor.matmul(
                                wq[:, 512 * hh : 512 * (hh + 1)], lhsT=hT[:, :],
                                rhs=w2pe_sb[
                                    :, 1024 * q + 512 * hh : 1024 * q + 512 * (hh + 1)
                                ],
                                start=True, stop=True,
                            )
                        if q < 3:
                            nc.scalar.activation(
                                w_ab[:, 1024 * q : 1024 * (q + 1)], wq[:], AF.Copy
                            )
                        else:
                            nc.scalar.activation(w_cd[:], wq[:], AF.Copy)
                    xcat = xgb[:, XT_COLS * t : XT_COLS * t + 64]
                    xs = xgb[:, XT_COLS * t : XT_COLS * t + 48]
                    xv = xgb[:, XT_COLS * t + XV_OFF : XT_COLS * t + XV_OFF + 48]
                    # ---- A+B merged path: 4x mult, 2 tree folds, reduce ----
                    m_t = msgp.tile([128, 96], F16, tag="m")
                    prodAB = workp.tile([128, 3072], F16, tag="prodAB")
                    pf1 = workp.tile([128, 1536], F16, tag="pf1")
                    pf2 = workp.tile([128, 768], F16, tag="pf2")
                    nc.vector.scalar_tensor_tensor(
                        out=prodAB[:].rearrange("p (o i) -> p o i", i=64),
                        in0=w_ab[:].rearrange("p (o i) -> p o i", i=64),
                        scalar=1.0,
                        in1=xcat.rearrange("p (o i) -> p o i", o=1)
                        .to_broadcast([128, 48, 64]),
                        op0=ALU.mult, op1=ALU.mult,
                    )
                    pAB = prodAB[:].rearrange("p (o i) -> p o i", i=64)
                    nc.vector.tensor_tensor(
                        out=pf1[:].rearrange("p (o i) -> p o i", i=32),
                        in0=pAB[:, :, 0:32], in1=pAB[:, :, 32:64], op=ALU.add,
                    )
                    pF1 = pf1[:].rearrange("p (o i) -> p o i", i=32)
                    nc.vector.tensor_tensor(
                        out=pf2[:].rearrange("p (o i) -> p o i", i=16),
                        in0=pF1[:, :, 0:16], in1=pF1[:, :, 16:32], op=ALU.add,
                    )
                    nc.vector.reduce_sum(
                        m_t[:, 0:48],
                        pf2[:].rearrange("p (o i) -> p o i", i=16),
                        axis=mybir.AxisListType.X,
                    )
                    # ---- C path ----
                    prodC = workp.tile([128, 768], F16, tag="prodC")
                    zC = msgp.tile([128, 16], F16, tag="zC")
                    nc.vector.scalar_tensor_tensor(
                        out=prodC[:].rearrange("p (o i) -> p o i", i=48),
                        in0=w_cd[:, 0:768].rearrange("p (o i) -> p o i", i=48),
                        scalar=1.0,
                        in1=xs.rearrange("p (o i) -> p o i", o=1)
                        .to_broadcast([128, 16, 48]),
                        op0=ALU.mult, op1=ALU.mult,
                    )
                    nc.vector.reduce_sum(
                        zC[:],
                        prodC[:].rearrange("p (o i) -> p o i", i=48),
                        axis=mybir.AxisListType.X,
                    )
                    # ---- D path (xv is (m,i) m-major; packed strided views) --
                    prodD = workp.tile([128, 768], F16, tag="prodD")
                    mvD = msgp.tile([128, 48], F16, tag="mvD")
                    wd = w_cd[:, 768:1024]
                    wd_omi = AP(wd.tensor, wd.offset,
                                [wd.ap[0], [16, 16], [0, 3], [1, 16]])
                    xv_omi = AP(xv.tensor, xv.offset,
                                [xv.ap[0], [0, 16], [16, 3], [1, 16]])
                    nc.vector.tensor_tensor(
                        out=prodD[:].rearrange("p (o m i) -> p o m i", m=3, i=16),
                        in0=wd_omi, in1=xv_omi, op=ALU.mult,
                    )
                    nc.vector.reduce_sum(
                        mvD[:].rearrange("p (o m) -> p o m", m=3),
                        prodD[:].rearrange("p (o m i) -> p o m i", m=3, i=16),
                        axis=mybir.AxisListType.X,
                    )
                    # ---- mv = zC x Y1 + mvD ----
                    y1t = y1b[:, 3 * t : 3 * t + 3]
                    mvC = msgp.tile([128, 48], F16, tag="mvC")
                    nc.vector.tensor_tensor(
                        out=mvC[:].rearrange("p (o m) -> p o m", m=3),
                        in0=zC[:].rearrange("p (o m) -> p o m", m=1).to_broadcast(
                            [128, 16, 3]
                        ),
                        in1=y1t.rearrange("p (o m) -> p o m", o=1).to_broadcast(
                            [128, 16, 3]
                        ),
                        op=ALU.mult,
                    )
                    nc.vector.tensor_tensor(
                        out=m_t[:, 48:96], in0=mvC[:], in1=mvD[:], op=ALU.add
                    )
                    # ---- scatter via one-hot matmuls, accumulate in PSUM ----
                    nc.tensor.matmul(
                        sc_s[:], lhsT=m_t[:, 0:48],
                        rhs=ohb[:, 128 * t : 128 * (t + 1)],
                        start=(t == 0), stop=(t == tiles_per_bucket - 1),
                    )
                    nc.tensor.matmul(
                        sc_v[:], lhsT=m_t[:, 48:96],
                        rhs=ohb[:, 128 * t : 128 * (t + 1)],
                        start=(t == 0), stop=(t == tiles_per_bucket - 1),
                    )
                # ---- node stage for bucket b (all transposed [feat, node]) ----
                acc_s = accp.tile([48, 128], F16, tag="accs_sb")
                acc_v = accp.tile([48, 128], F16, tag="accv_sb")
                nc.scalar.activation(acc_s[:], sc_s[:], AF.Copy)
                nc.scalar.activation(acc_v[:], sc_v[:], AF.Copy)
                node_ps = pnp.tile([128, 512], FP, tag="node")
                sT_ps = node_ps[0:48, 0:128]
                gT_ps = node_ps[0:48, 128:256]
                nsT_ps = node_ps[0:48, 256:384]
                nc.tensor.matmul(
                    sT_ps, lhsT=ws_sb[:], rhs=acc_s[:], start=True, stop=True
                )
                nc.tensor.matmul(
                    gT_ps, lhsT=wg_sb[:], rhs=acc_s[:], start=True, stop=True
                )
                nc.tensor.matmul(
                    nsT_ps, lhsT=wns_sb[:], rhs=acc_v[:], start=True, stop=True
                )
                # gate: sigmoid(z) = (tanh(z/2)+1)/2; the /2 factors are
                # folded host-side into wg (z/2) and wns (ns/2)
                sT = msgp.tile([48, 128], F16, tag="sT_sb")
                gT = msgp.tile([48, 128], F16, tag="gT_sb")
                fin = msgp.tile([48, 128], FP, tag="fin")
                nc.scalar.activation(sT[:], sT_ps, AF.Silu)
                nc.scalar.activation(gT[:], gT_ps, AF.Tanh)
                nc.vector.scalar_tensor_tensor(
                    out=fin[:], in0=gT[:], scalar=1.0, in1=nsT_ps,
                    op0=ALU.add, op1=ALU.mult,
                )
                nc.vector.tensor_tensor(
                    out=fin[:], in0=fin[:], in1=sT[:], op=ALU.add
                )
                finT_ps = node_ps[:, 384:432]
                nc.tensor.transpose(finT_ps, fin[:], identf_sb[:48, :48])
                fino = msgp.tile([128, 48], FP, tag="fino")
                nc.scalar.activation(fino[:], finT_ps, AF.Copy)
                nc.sync.dma_start(
                    out=d_out[128 * b : 128 * (b + 1), :], in_=fino[:]
                )
          if rep_ctx is not None:
              rep_ctx.__exit__(None, None, None)
    lp_ctx.__exit__(None, None, None)
    nc.finalize()
    return nc


def _make_in_maps(inputs, srcidx, dstpos, dstloc):
    x = np.asarray(inputs["x"], np.float32)
    pos = np.asarray(inputs["pos"], np.float32)
    xtab = np.zeros((N_NODES, XT_COLS), np.float32)
    xtab[:, XS_OFF : XS_OFF + 48] = x[:, :48]
    # xv stored m-major: col XV_OFF + m*16 + i = x[:, 48 + i*3 + m]
    xv = x[:, 48:96].reshape(N_NODES, 16, 3)
    xtab[:, XV_OFF : XV_OFF + 48] = xv.transpose(0, 2, 1).reshape(N_NODES, 48)
    hi32 = pos.astype(np.float16).astype(np.float32)
    lo = (pos - hi32).astype(np.float32)
    xtab[:, PH_OFF : PH_OFF + 3] = hi32
    xtab[:, PL_OFF : PL_OFF + 3] = lo
    xtab_f16 = xtab.astype(np.float16)
    ptab = np.zeros((N_NODES, 4), np.float32)
    ptab[:, 0:3] = pos

    htab = _make_htab(np.asarray(inputs["w1"], np.float32).reshape(-1))
    w2pe = _make_w2pe(np.asarray(inputs["w2"], np.float32)).astype(np.float16)
    ws_c = (np.asarray(inputs["Ws"], np.float32) / np.sqrt(M0)).astype(np.float16)
    # /2 for the tanh-based sigmoid gate: sigmoid(z) = (tanh(z/2)+1)/2
    wg_c = (np.asarray(inputs["Wg"], np.float32) / (2.0 * np.sqrt(M0))).astype(
        np.float16
    )
    wns_c = (0.5 * _wns_block(np.asarray(inputs["Wns"], np.float32))).astype(
        np.float16
    )
    identf = np.eye(128, dtype=np.float32)
    iota = np.tile(np.arange(128, dtype=np.float32), (128, 1)).astype(np.float16)
    in_maps = []
    for c in range(N_CORES):
        in_maps.append({
            "xtab": xtab_f16, "ptab": ptab, "htab": htab,
            "srcidx": np.ascontiguousarray(srcidx[c]),
            "dstpos": np.ascontiguousarray(dstpos[c]),
            "dstloc": np.ascontiguousarray(dstloc[c]).astype(np.float16),
            "w2pe": w2pe, "ws": ws_c, "wg": wg_c, "wns": wns_c,
            "identf": identf, "iota": iota,
        })
    return in_maps


def kernel(x, pos, edge_index, w1, w2, Ws, Wns, Wg):
    inputs = {"x": x, "pos": pos, "w1": w1, "w2": w2,
              "Ws": Ws, "Wns": Wns, "Wg": Wg}
    srcidx, dstpos, dstloc, tiles_per_bucket = _prep_edges(
        np.asarray(edge_index, np.int64)
    )
    in_maps = _make_in_maps(inputs, srcidx, dstpos, dstloc)
    nc = build_kernel(tiles_per_bucket)
    res = run_bass_kernel_spmd(nc, in_maps, core_ids=list(range(N_CORES)))
    return np.concatenate([res.results[c]["out"] for c in range(N_CORES)], axis=0)


# revision 47
# speedup vs baseline: 1.0456x; 1.0456x over previous
"""EquivariantEdgeConv fused Bass kernel for one TRN2 chip (8 NeuronCores).

Strategy (node-sharded scatter, edge-bucketed message passing), v4:
  - Nodes are sharded: core c owns nodes [1024c, 1024c+1024), i.e. 8
    buckets of 128 nodes each. Each core receives exactly the edges whose
    *destination* lands in its node range, grouped by 128-node bucket and
    padded per-bucket to a multiple of 128 (shared static capacity).
  - The radial MLP's sqrt + silu are replaced by an 8192-entry lookup
    table over lensq with a Mobius index warp idx = 8192*q/(q+2)
    (= 8192 - 16384/(q+2): one reciprocal + one fused scale-add on DVE).
    Each row holds h = silu(w1*len) (64 fp32) plus 1/len, so the only
    ACT functions are Copy (+ Silu/Tanh per bucket) - one table set,
    zero activation-table reloads.
  - GPSIMD runs NO tensor ops (firmware tensor ops are very slow on HW);
    it only issues the indirect gathers, and those are batched per
    BUCKET (one gather per table per bucket via [128,T] offset APs),
    amortizing the ~1us SWDGE fixed cost 9x.
  - All per-edge elementwise work is fp16 (2-byte packed, SBUF) so the
    multiplies run in the DVE's 4x TensorScalarPtr mode; the A+B path
    reduce runs as two 2x tree-folds + one short reduce. fp16 also
    roughly halves the rounding error vs bf16.
  - Per 128-edge tile: PE does the h transpose, 8 radial matmul halves
    into PSUM quarters, and 2 one-hot scatter matmuls; ACT copies the
    quarters to SBUF fp16; DVE does the TP mults/reduces and combines.
  - Per bucket, the gated output block (o3.Linear + silu / sigmoid-via-
    tanh gate) runs transposed on PE/ACT/DVE and is DMA'd out.
    Outputs concatenate across cores - no collective needed.

Self-contained: shapes/sharding hardcoded for N=8192 nodes, E=65536
edges, irreps 48x0e + 16x1o, H=64.
"""

import sys

if "/opt/trn_rl_repo" not in sys.path:
    sys.path.insert(0, "/opt/trn_rl_repo")

import numpy as np

import concourse.bacc as bacc
import concourse.bass as bass
import concourse.mybir as mybir
import concourse.tile as tile
from concourse.bass import AP
from concourse.bass_utils import run_bass_kernel_spmd

M0, M1, H = 48, 16, 64
N_NODES, N_EDGES, N_CORES = 8192, 65536, 8
NODES_PER_CORE = N_NODES // N_CORES          # 1024
BUCKETS = NODES_PER_CORE // 128              # 8 buckets of 128 nodes per core
FP = mybir.dt.float32
F16 = mybir.dt.float16
I32 = mybir.dt.int32

# path normalizations (cA..cD) and the radial-MLP 1/sqrt(H), folded into w2
CA = 1.0 / np.sqrt(M0 * 2.0)
CB = 1.0 / np.sqrt(3.0 * M1 * 2.0)
CC = 1.0 / np.sqrt(M0 * 2.0)
CD = 1.0 / np.sqrt(M1 * 2.0)
SQRT3 = float(np.sqrt(3.0))

# xtab column layout (fp16): [xs 48 | xvy 16 (device-filled) | xv 48
# m-major (m,i) | pos_hi 3 | pos_lo 3 | pad 2] = 120
XS_OFF, XVY_OFF, XV_OFF, PH_OFF, PL_OFF, XT_COLS = 0, 48, 64, 112, 115, 120

# radial lookup table: HTAB rows over q = lensq, warp w(q) = W*q/(q+K)
HTAB_N, HTAB_K = 8192, 2.0
HTAB_COLS = 72  # [h 64 | 1/len | pad]

# w2pe column layout [64, 4096] (fp16, scales folded in):
#   AB block [0, 3072): col o*64 + i, o in 48; i<48 -> A path (i), i>=48 ->
#     B path (i-48).  One DVE mult+fold+reduce covers A+B -> ms[48].
#   C block [3072, 3840): col 3072 + o*48 + i, o in 16, i in 48.
#   D block [3840, 4096): col 3840 + o*16 + i, o,i in 16.
AB_OFF, C_OFF, D_OFF = 0, 3072, 3840


def _make_w2pe(w2: np.ndarray) -> np.ndarray:
    inv_sqrt_h = 1.0 / np.sqrt(H)
    perm = np.empty(4096, np.int64)
    scale = np.empty(4096, np.float32)
    for o in range(48):
        for i in range(48):
            perm[AB_OFF + o * 64 + i] = i * 48 + o
            scale[AB_OFF + o * 64 + i] = CA * inv_sqrt_h
        for ib in range(16):
            perm[AB_OFF + o * 64 + 48 + ib] = 2304 + ib * 48 + o
            scale[AB_OFF + o * 64 + 48 + ib] = CB * SQRT3 * inv_sqrt_h
    for o in range(16):
        for i in range(48):
            perm[C_OFF + o * 48 + i] = 3072 + i * 16 + o
            scale[C_OFF + o * 48 + i] = CC * SQRT3 * inv_sqrt_h
        for i in range(16):
            perm[D_OFF + o * 16 + i] = 3840 + i * 16 + o
            scale[D_OFF + o * 16 + i] = CD * inv_sqrt_h
    return (w2[:, perm] * scale[None, :]).astype(np.float32)


def _make_htab(w1: np.ndarray) -> np.ndarray:
    """[HTAB_N, HTAB_COLS] fp32: row i covers q near q_i = K*w/(W-w) with
    w = i+0.5; cols 0:64 = silu(w1*len), col 64 = 1/len."""
    i = np.arange(HTAB_N, dtype=np.float64)
    w = i + 0.5
    q = HTAB_K * w / (HTAB_N - w)
    length = np.sqrt(q)
    z = length[:, None] * np.asarray(w1, np.float64).reshape(1, H)
    h = z / (1.0 + np.exp(-z))
    tab = np.zeros((HTAB_N, HTAB_COLS), np.float32)
    tab[:, 0:64] = h.astype(np.float32)
    tab[:, 64] = (1.0 / length).astype(np.float32)
    return tab.astype(np.float16)


def _wns_block(wns: np.ndarray) -> np.ndarray:
    """[48,48] lhsT for the 1o o3.Linear on (o,m)-interleaved rows:
    lhsT[(i,m),(o,m')] = Wns[i,o] * delta(m,m') / sqrt(M1)."""
    out = np.zeros((48, 48), np.float32)
    for i in range(16):
        for m in range(3):
            for o in range(16):
                out[i * 3 + m, o * 3 + m] = wns[i, o] / np.sqrt(M1)
    return out


def _prep_edges(edge_index: np.ndarray):
    """Bucket/pad edges by destination. Returns per-core index arrays and
    the shared per-bucket tile count."""
    src, dst = edge_index[0].astype(np.int64), edge_index[1].astype(np.int64)
    gb = dst >> 7  # global bucket 0..63
    order = np.argsort(gb, kind="stable")
    src_s, dst_s, gb_s = src[order], dst[order], gb[order]
    counts = np.bincount(gb_s, minlength=64)
    cap = int(np.ceil(counts.max() / 128) * 128)
    tiles_per_bucket = cap // 128

    srcidx = np.zeros((N_CORES, BUCKETS * cap), np.int32)
    dstpos = np.zeros((N_CORES, BUCKETS * cap), np.int32)
    dstloc = np.full((N_CORES, BUCKETS * cap), 300.0, np.float32)
    starts = np.concatenate([[0], np.cumsum(counts)])
    for g in range(64):
        c, b = g >> 3, g & 7
        s, e = starts[g], starts[g + 1]
        n = e - s
        o = b * cap
        srcidx[c, o : o + n] = src_s[s:e]
        dstpos[c, o : o + n] = dst_s[s:e]
        dstloc[c, o : o + n] = (dst_s[s:e] - (g << 7)).astype(np.float32)
    # reshape to [BUCKETS*128, T]: column t = tile t's per-partition indices
    def to_cols(a):
        out = np.empty((N_CORES, BUCKETS * 128, tiles_per_bucket), a.dtype)
        for b in range(BUCKETS):
            blk = a[:, b * cap : (b + 1) * cap].reshape(N_CORES, tiles_per_bucket, 128)
            out[:, b * 128 : (b + 1) * 128, :] = blk.transpose(0, 2, 1)
        return out
    return to_cols(srcidx), to_cols(dstpos), to_cols(dstloc), tiles_per_bucket


def build_kernel(tiles_per_bucket: int, reps: int = 1) -> bass.Bass:
    nc = bacc.Bacc(None, target_bir_lowering=False, debug=False)
    d_xtab = nc.declare_dram_parameter("xtab", [N_NODES, XT_COLS], F16, isOutput=False)
    d_ptab = nc.declare_dram_parameter("ptab", [N_NODES, 4], FP, isOutput=False)
    d_htab = nc.declare_dram_parameter("htab", [HTAB_N, HTAB_COLS], F16, isOutput=False)
    T = tiles_per_bucket
    d_srcidx = nc.declare_dram_parameter("srcidx", [BUCKETS * 128, T], I32, isOutput=False)
    d_dstpos = nc.declare_dram_parameter("dstpos", [BUCKETS * 128, T], I32, isOutput=False)
    d_dstloc = nc.declare_dram_parameter("dstloc", [BUCKETS * 128, T], F16, isOutput=False)
    d_w2pe = nc.declare_dram_parameter("w2pe", [H, 4096], F16, isOutput=False)
    d_ws = nc.declare_dram_parameter("ws", [M0, M0], F16, isOutput=False)
    d_wg = nc.declare_dram_parameter("wg", [M0, M0], F16, isOutput=False)
    d_wns = nc.declare_dram_parameter("wns", [48, 48], F16, isOutput=False)
    d_identf = nc.declare_dram_parameter("identf", [128, 128], FP, isOutput=False)
    d_iota = nc.declare_dram_parameter("iota", [128, 128], F16, isOutput=False)
    d_out = nc.declare_dram_parameter("out", [NODES_PER_CORE, M0], FP, isOutput=True)

    AF = mybir.ActivationFunctionType
    ALU = mybir.AluOpType

    lp_ctx = nc.allow_low_precision(
        reason="fp16 TP reduces over <=64 terms; rel-err gate is 2e-2"
    )
    lp_ctx.__enter__()
    with tile.TileContext(nc) as tc, tc.tile_pool(name="consts", bufs=1) as cp:
        w2pe_sb = cp.tile([H, 4096], F16)
        ws_sb = cp.tile([M0, M0], F16)
        wg_sb = cp.tile([M0, M0], F16)
        wns_sb = cp.tile([48, 48], F16)
        identf_sb = cp.tile([128, 128], FP)
        iota_sb = cp.tile([128, 128], F16)
        for sb, dr in (
            (w2pe_sb, d_w2pe), (ws_sb, d_ws), (wg_sb, d_wg),
            (wns_sb, d_wns), (identf_sb, d_identf), (iota_sb, d_iota),
        ):
            nc.sync.dma_start(out=sb[:], in_=dr[:])

        with (
            tc.tile_pool(name="idx", bufs=2) as idxp,
            tc.tile_pool(name="gath", bufs=2) as gathp,
            tc.tile_pool(name="geo", bufs=2) as geop,
            tc.tile_pool(name="work", bufs=2) as workp,
            tc.tile_pool(name="msg", bufs=2) as msgp,
            tc.tile_pool(name="accs", bufs=1) as accp,
            tc.tile_pool(name="accps", bufs=1, space="PSUM") as accpp,
            tc.tile_pool(name="wps", bufs=2, space="PSUM") as wpsp,
            tc.tile_pool(name="ps_small", bufs=1, space="PSUM") as psp,
            tc.tile_pool(name="ps_node", bufs=1, space="PSUM") as pnp,
        ):
          rep_ctx = tc.For_i(0, reps, 1) if reps > 1 else None
          if rep_ctx is not None:
              rep_ctx.__enter__()
          if True:
            for b in range(BUCKETS):
                sc_s = accpp.tile([48, 128], FP, tag="acc_s")
                sc_v = accpp.tile([48, 128], FP, tag="acc_v")
                bidx_s = idxp.tile([128, T], I32, tag="idx_s")
                bidx_d = idxp.tile([128, T], I32, tag="idx_d")
                bdl = idxp.tile([128, T], F16, tag="dl")
                nc.sync.dma_start(out=bidx_s[:], in_=d_srcidx[128 * b : 128 * (b + 1), :])
                nc.sync.dma_start(out=bidx_d[:], in_=d_dstpos[128 * b : 128 * (b + 1), :])
                nc.sync.dma_start(out=bdl[:], in_=d_dstloc[128 * b : 128 * (b + 1), :])
                # ---- per-tile gathers into bucket-wide tiles (HW uses
                # only the first offset per partition, so one gather per
                # tile; compute stays bucket-batched over the slices) ----
                xgb = gathp.tile([128, XT_COLS * T], F16, tag="xgb")
                pdb = gathp.tile([128, 4 * T], FP, tag="pdb")
                for t in range(T):
                    nc.gpsimd.indirect_dma_start(
                        out=xgb[:, XT_COLS * t : XT_COLS * (t + 1)],
                        out_offset=None, in_=d_xtab[:],
                        in_offset=bass.IndirectOffsetOnAxis(
                            ap=bidx_s[:, t : t + 1], axis=0),
                    )
                    nc.gpsimd.indirect_dma_start(
                        out=pdb[:, 4 * t : 4 * (t + 1)],
                        out_offset=None, in_=d_ptab[:],
                        in_offset=bass.IndirectOffsetOnAxis(
                            ap=bidx_d[:, t : t + 1], axis=0),
                    )
                # ---- bucket-batched geometry + radial-table index ----
                vecb = geop.tile([128, 3 * T], FP, tag="vecb")
                sqb = geop.tile([128, 3 * T], FP, tag="sqb")
                qkb = geop.tile([128, T], FP, tag="qkb")
                rkb = geop.tile([128, T], FP, tag="rkb")
                hidxb = geop.tile([128, T], I32, tag="hidxb")
                y1b = geop.tile([128, 3 * T], F16, tag="y1b")
                pvyb = geop.tile([128, 48 * T], F16, tag="pvyb")
                ohb = geop.tile([128, 128 * T], F16, tag="ohb")

                def v3ap(t_ap, stride, off=0):
                    return AP(t_ap.tensor, t_ap.offset + off,
                              [t_ap.ap[0], [stride, T], [1, 3]])
                # vec = pd - (pos_hi + pos_lo) of src, in fp32
                nc.vector.tensor_tensor(
                    out=v3ap(vecb[:], 3), in0=v3ap(pdb[:], 4),
                    in1=v3ap(xgb[:], XT_COLS, PH_OFF), op=ALU.subtract,
                )
                nc.vector.tensor_tensor(
                    out=v3ap(vecb[:], 3), in0=v3ap(vecb[:], 3),
                    in1=v3ap(xgb[:], XT_COLS, PL_OFF), op=ALU.subtract,
                )
                nc.vector.tensor_tensor(
                    out=v3ap(sqb[:], 3), in0=v3ap(vecb[:], 3),
                    in1=v3ap(vecb[:], 3), op=ALU.mult,
                )
                nc.vector.reduce_sum(
                    qkb[:], sqb[:].rearrange("p (t c) -> p t c", c=3),
                    axis=mybir.AxisListType.X,
                )
                nc.vector.tensor_scalar_add(qkb[:], qkb[:], HTAB_K)
                nc.vector.reciprocal(rkb[:], qkb[:])
                hfb = geop.tile([128, T], FP, tag="hfb")
                nc.vector.tensor_scalar(
                    out=hfb[:], in0=rkb[:],
                    scalar1=-float(HTAB_N * HTAB_K), scalar2=float(HTAB_N),
                    op0=ALU.mult, op1=ALU.add,
                )
                nc.vector.tensor_scalar_max(hfb[:], hfb[:], 0.0)
                nc.vector.tensor_scalar_min(hfb[:], hfb[:], float(HTAB_N - 1))
                nc.vector.tensor_copy(hidxb[:], hfb[:])
                hgb = gathp.tile([128, HTAB_COLS * T], F16, tag="hgb")
                for t in range(T):
                    nc.gpsimd.indirect_dma_start(
                        out=hgb[:, HTAB_COLS * t : HTAB_COLS * (t + 1)],
                        out_offset=None, in_=d_htab[:],
                        in_offset=bass.IndirectOffsetOnAxis(
                            ap=hidxb[:, t : t + 1], axis=0),
                    )
                # y1 = vec / len
                invl = AP(hgb.tensor, hgb.offset + 64,
                          [hgb[:].ap[0], [HTAB_COLS, T], [0, 3]])
                nc.vector.tensor_tensor(
                    out=v3ap(y1b[:], 3), in0=v3ap(vecb[:], 3), in1=invl,
                    op=ALU.mult,
                )
                # pvy[t,(m,i)] = xv[t,(m,i)] * y1[t,m]
                xv_tmi = AP(xgb.tensor, xgb.offset + XV_OFF,
                            [xgb[:].ap[0], [XT_COLS, T], [16, 3], [1, 16]])
                y1_tmi = AP(y1b.tensor, y1b.offset,
                            [y1b[:].ap[0], [3, T], [1, 3], [0, 16]])
                pvy_tmi = AP(pvyb.tensor, pvyb.offset,
                             [pvyb[:].ap[0], [48, T], [16, 3], [1, 16]])
                nc.vector.tensor_tensor(
                    out=pvy_tmi, in0=xv_tmi, in1=y1_tmi, op=ALU.mult
                )
                # xvy[t,i] = sum_m pvy[t,(m,i)] -> xgb cols [48,64) of tile t
                pvy_tim = AP(pvyb.tensor, pvyb.offset,
                             [pvyb[:].ap[0], [48, T], [1, 16], [16, 3]])
                xvy_out = AP(xgb.tensor, xgb.offset + XVY_OFF,
                             [xgb[:].ap[0], [XT_COLS, T], [1, 16]])
                nc.vector.tensor_reduce(
                    out=xvy_out, in_=pvy_tim, axis=mybir.AxisListType.X,
                    op=ALU.add,
                )
                # one-hot for all tiles: ohb[p,(t,n)] = (iota[n] == dl[p,t])
                iota_tn = AP(iota_sb.tensor, iota_sb.offset,
                             [iota_sb[:].ap[0], [0, T], [1, 128]])
                dl_tn = AP(bdl.tensor, bdl.offset,
                           [bdl[:].ap[0], [1, T], [0, 128]])
                nc.vector.tensor_tensor(
                    out=ohb[:].rearrange("p (t n) -> p t n", n=128),
                    in0=iota_tn, in1=dl_tn, op=ALU.is_equal,
                )
                for t in range(tiles_per_bucket):
                    # ---- hT = gathered h, transposed on PE ----
                    h32 = geop.tile([128, 64], FP, tag="h32")
                    nc.scalar.activation(
                        h32[:], hgb[:, HTAB_COLS * t : HTAB_COLS * t + 64],
                        AF.Copy,
                    )
                    hT_ps = psp.tile([64, 128], FP, tag="hT_ps")
                    nc.tensor.transpose(hT_ps[:], h32[:], identf_sb[:])
                    hT = geop.tile([H, 128], F16, tag="hT")
                    nc.scalar.activation(hT[:], hT_ps[:], AF.Copy)
                    # ---- radial matmul in 4 quarters; PSUM -> SBUF fp16 ----
                    w_ab = workp.tile([128, 3072], F16, tag="w_ab")
                    w_cd = workp.tile([128, 1024], F16, tag="w_cd")
                    for q in range(4):
                        wq = wpsp.tile([128, 1024], FP, tag="wq")
                        for hh in range(2):
                            nc.tensor.matmul(
                                wq[:, 512 * hh : 512 * (hh + 1)], lhsT=hT[:, :],
                                rhs=w2pe_sb[
                                    :, 1024 * q + 512 * hh : 1024 * q + 512 * (hh + 1)
                                ],
                                start=True, stop=True,
                            )
                        if q < 3:
                            nc.scalar.activation(
                                w_ab[:, 1024 * q : 1024 * (q + 1)], wq[:], AF.Copy
                            )
                        else:
                            nc.scalar.activation(w_cd[:], wq[:], AF.Copy)
                    xcat = xgb[:, XT_COLS * t : XT_COLS * t + 64]
                    xs = xgb[:, XT_COLS * t : XT_COLS * t + 48]
                    xv = xgb[:, XT_COLS * t + XV_OFF : XT_COLS * t + XV_OFF + 48]
                    # ---- A+B merged path: 4x mult, 2 tree folds, reduce ----
                    m_t = msgp.tile([128, 96], F16, tag="m")
                    prodAB = workp.tile([128, 3072], F16, tag="prodAB")
                    pf1 = workp.tile([128, 1536], F16, tag="pf1")
                    pf2 = workp.tile([128, 768], F16, tag="pf2")
                    nc.vector.scalar_tensor_tensor(
                        out=prodAB[:].rearrange("p (o i) -> p o i", i=64),
                        in0=w_ab[:].rearrange("p (o i) -> p o i", i=64),
                        scalar=1.0,
                        in1=xcat.rearrange("p (o i) -> p o i", o=1)
                        .to_broadcast([128, 48, 64]),
                        op0=ALU.mult, op1=ALU.mult,
                    )
                    pAB = prodAB[:].rearrange("p (o i) -> p o i", i=64)
                    nc.vector.tensor_tensor(
                        out=pf1[:].rearrange("p (o i) -> p o i", i=32),
                        in0=pAB[:, :, 0:32], in1=pAB[:, :, 32:64], op=ALU.add,
                    )
                    pF1 = pf1[:].rearrange("p (o i) -> p o i", i=32)
                    nc.vector.tensor_tensor(
                        out=pf2[:].rearrange("p (o i) -> p o i", i=16),
                        in0=pF1[:, :, 0:16], in1=pF1[:, :, 16:32], op=ALU.add,
                    )
                    nc.vector.reduce_sum(
                        m_t[:, 0:48],
                        pf2[:].rearrange("p (o i) -> p o i", i=16),
                        axis=mybir.AxisListType.X,
                    )
                    # ---- C path ----
                    prodC = workp.tile([128, 768], F16, tag="prodC")
                    zC = msgp.tile([128, 16], F16, tag="zC")
                    nc.vector.scalar_tensor_tensor(
                        out=prodC[:].rearrange("p (o i) -> p o i", i=48),
                        in0=w_cd[:, 0:768].rearrange("p (o i) -> p o i", i=48),
                        scalar=1.0,
                        in1=xs.rearrange("p (o i) -> p o i", o=1)
                        .to_broadcast([128, 16, 48]),
                        op0=ALU.mult, op1=ALU.mult,
                    )
                    nc.vector.reduce_sum(
                        zC[:],
                        prodC[:].rearrange("p (o i) -> p o i", i=48),
                        axis=mybir.AxisListType.X,
                    )
                    # ---- D path (xv is (m,i) m-major; packed strided views) --
                    prodD = workp.tile([128, 768], F16, tag="prodD")
                    mvD = msgp.tile([128, 48], F16, tag="mvD")
                    wd = w_cd[:, 768:1024]
                    wd_omi = AP(wd.tensor, wd.offset,
                                [wd.ap[0], [16, 16], [0, 3], [1, 16]])
                    xv_omi = AP(xv.tensor, xv.offset,
                                [xv.ap[0], [0, 16], [16, 3], [1, 16]])
                    nc.vector.tensor_tensor(
                        out=prodD[:].rearrange("p (o m i) -> p o m i", m=3, i=16),
                        in0=wd_omi, in1=xv_omi, op=ALU.mult,
                    )
                    nc.vector.reduce_sum(
                        mvD[:].rearrange("p (o m) -> p o m", m=3),
                        prodD[:].rearrange("p (o m i) -> p o m i", m=3, i=16),
                        axis=mybir.AxisListType.X,
                    )
                    # ---- mv = zC x Y1 + mvD ----
                    y1t = y1b[:, 3 * t : 3 * t + 3]
                    mvC = msgp.tile([128, 48], F16, tag="mvC")
                    nc.vector.tensor_tensor(
                        out=mvC[:].rearrange("p (o m) -> p o m", m=3),
                        in0=zC[:].rearrange("p (o m) -> p o m", m=1).to_broadcast(
                            [128, 16, 3]
                        ),
                        in1=y1t.rearrange("p (o m) -> p o m", o=1).to_broadcast(
                            [128, 16, 3]
                        ),
                        op=ALU.mult,
                    )
                    nc.vector.tensor_tensor(
                        out=m_t[:, 48:96], in0=mvC[:], in1=mvD[:], op=ALU.add
                    )
                    # ---- scatter via one-hot matmuls, accumulate in PSUM ----
                    nc.tensor.matmul(
                        sc_s[:], lhsT=m_t[:, 0:48],
                        rhs=ohb[:, 128 * t : 128 * (t + 1)],
                        start=(t == 0), stop=(t == tiles_per_bucket - 1),
                    )
                    nc.tensor.matmul(
                        sc_v[:], lhsT=m_t[:, 48:96],
                        rhs=ohb[:, 128 * t : 128 * (t + 1)],
                        start=(t == 0), stop=(t == tiles_per_bucket - 1),
                    )
                # ---- node stage for bucket b (all transposed [feat, node]) ----
                acc_s = accp.tile([48, 128], F16, tag="accs_sb")
                acc_v = accp.tile([48, 128], F16, tag="accv_sb")
                nc.scalar.activation(acc_s[:], sc_s[:], AF.Copy)
                nc.scalar.activation(acc_v[:], sc_v[:], AF.Copy)
                node_ps = pnp.tile([128, 512], FP, tag="node")
                sT_ps = node_ps[0:48, 0:128]
                gT_ps = node_ps[0:48, 128:256]
                nsT_ps = node_ps[0:48, 256:384]
                nc.tensor.matmul(
                    sT_ps, lhsT=ws_sb[:], rhs=acc_s[:], start=True, stop=True
                )
                nc.tensor.matmul(
                    gT_ps, lhsT=wg_sb[:], rhs=acc_s[:], start=True, stop=True
                )
                nc.tensor.matmul(
                    nsT_ps, lhsT=wns_sb[:], rhs=acc_v[:], start=True, stop=True
                )
                # gate: sigmoid(z) = (tanh(z/2)+1)/2; the /2 factors are
                # folded host-side into wg (z/2) and wns (ns/2)
                sT = msgp.tile([48, 128], F16, tag="sT_sb")
                gT = msgp.tile([48, 128], F16, tag="gT_sb")
                fin = msgp.tile([48, 128], FP, tag="fin")
                nc.scalar.activation(sT[:], sT_ps, AF.Silu)
                nc.scalar.activation(gT[:], gT_ps, AF.Tanh)
                nc.vector.scalar_tensor_tensor(
                    out=fin[:], in0=gT[:], scalar=1.0, in1=nsT_ps,
                    op0=ALU.add, op1=ALU.mult,
                )
                nc.vector.tensor_tensor(
                    out=fin[:], in0=fin[:], in1=sT[:], op=ALU.add
                )
                finT_ps = node_ps[:, 384:432]
                nc.tensor.transpose(finT_ps, fin[:], identf_sb[:48, :48])
                fino = msgp.tile([128, 48], FP, tag="fino")
                nc.scalar.activation(fino[:], finT_ps, AF.Copy)
                nc.sync.dma_start(
                    out=d_out[128 * b : 128 * (b + 1), :], in_=fino[:]
                )
          if rep_ctx is not None:
              rep_ctx.__exit__(None, None, None)
    lp_ctx.__exit__(None, None, None)
    nc.finalize()
    return nc


def _make_in_maps(inputs, srcidx, dstpos, dstloc):
    x = np.asarray(inputs["x"], np.float32)
    pos = np.asarray(inputs["pos"], np.float32)
    xtab = np.zeros((N_NODES, XT_COLS), np.float32)
    xtab[:, XS_OFF : XS_OFF + 48] = x[:, :48]
    # xv stored m-major: col XV_OFF + m*16 + i = x[:, 48 + i*3 + m]
    xv = x[:, 48:96].reshape(N_NODES, 16, 3)
    xtab[:, XV_OFF : XV_OFF + 48] = xv.transpose(0, 2, 1).reshape(N_NODES, 48)
    hi32 = pos.astype(np.float16).astype(np.float32)
    lo = (pos - hi32).astype(np.float32)
    xtab[:, PH_OFF : PH_OFF + 3] = hi32
    xtab[:, PL_OFF : PL_OFF + 3] = lo
    xtab_f16 = xtab.astype(np.float16)
    ptab = np.zeros((N_NODES, 4), np.float32)
    ptab[:, 0:3] = pos

    htab = _make_htab(np.asarray(inputs["w1"], np.float32).reshape(-1))
    w2pe = _make_w2pe(np.asarray(inputs["w2"], np.float32)).astype(np.float16)
    ws_c = (np.asarray(inputs["Ws"], np.float32) / np.sqrt(M0)).astype(np.float16)
    # /2 for the tanh-based sigmoid gate: sigmoid(z) = (tanh(z/2)+1)/2
    wg_c = (np.asarray(inputs["Wg"], np.float32) / (2.0 * np.sqrt(M0))).astype(
        np.float16
    )
    wns_c = (0.5 * _wns_block(np.asarray(inputs["Wns"], np.float32))).astype(
        np.float16
    )
    identf = np.eye(128, dtype=np.float32)
    iota = np.tile(np.arange(128, dtype=np.float32), (128, 1)).astype(np.float16)
    in_maps = []
    for c in range(N_CORES):
        in_maps.append({
            "xtab": xtab_f16, "ptab": ptab, "htab": htab,
            "srcidx": np.ascontiguousarray(srcidx[c]),
            "dstpos": np.ascontiguousarray(dstpos[c]),
            "dstloc": np.ascontiguousarray(dstloc[c]).astype(np.float16),
            "w2pe": w2pe, "ws": ws_c, "wg": wg_c, "wns": wns_c,
            "identf": identf, "iota": iota,
        })
    return in_maps


def kernel(x, pos, edge_index, w1, w2, Ws, Wns, Wg):
    inputs = {"x": x, "pos": pos, "w1": w1, "w2": w2,
              "Ws": Ws, "Wns": Wns, "Wg": Wg}
    srcidx, dstpos, dstloc, tiles_per_bucket = _prep_edges(
        np.asarray(edge_index, np.int64)
    )
    in_maps = _make_in_maps(inputs, srcidx, dstpos, dstloc)
    nc = build_kernel(tiles_per_bucket)
    res = run_bass_kernel_spmd(nc, in_maps, core_ids=list(range(N_CORES)))
    return np.concatenate([res.results[c]["out"] for c in range(N_CORES)], axis=0)
